# revision 20
# baseline (speedup 1.0000x reference)
"""Trainium2 Bass kernel for nn_NSRLossV2 (8-core SPMD).

Math (reference, fp32):
    a  = x @ W1 + b1            [B, H]
    h  = relu(a)
    z  = h @ W2 + b2            [B, C]
    mse    = mean((z - onehot(y))^2)
    margin = sum(relu(1 - z_y + z) * (1 - onehot)) / B
    grads  = ((a > 0) * W2[:, y].T) @ W1.T        [B, D]
    w_l1   = sum_d |grads|
    R      = w_l1 * EPS / (|z_y| + 1e-8)
    nsr    = BETA * mean(log(1 + R))
    acc    = mean(argmax(z) == y)
    loss   = mse + (margin + nsr) * acc

Sharding: model-parallel over H (each core owns a 512-wide slice of W1's
columns).  mm1 in bf16: the first k-half runs k-group-major so the PE
tracks the xt DMA stream, the second half m-major so each 128-row tile
of G = (a>0)*W2[:,y] completes early.  G is cast to fp8 e4m3 (x1024) and
AllGathered in two chunks overlapped with mm1's tail and mm3; fp32 z
partials ride chunk B bitcast into extra byte columns.  mm3 (grads) runs
in fp8 DoubleRow perf mode (2 k-tiles per matmul).  Per-sample |grads|
L1 partials are AllReduced at the end.
"""

import os
import functools

import numpy as np
import ml_dtypes

import concourse.bass as bass
import concourse.bacc as bacc
import concourse.mybir as mybir
import concourse.tile as tile
from concourse.bass_utils import run_bass_kernel_spmd

NCORES = 8
B, D, H, C = 512, 4096, 4096, 5
HC = H // NCORES          # per-core H slice (512)
DC = D // NCORES          # per-core D slice (512)
P = 128
KT = D // P               # 32 k-tiles over D (mm1)
KG = 4                    # k-tiles per grouped xt DMA
NG = KT // KG             # 8 grouped xt loads
MT = HC // P              # 4 m-tiles over the local H slice
BT = B // P               # 4 b-tiles over the batch
BETA, EPS = 0.4, 0.05
GSCALE = 1024.0           # fp8 scale for G and W1T (grads scaled by 2^20)
EPS_EFF = EPS / (GSCALE * GSCALE)

# AllGather chunking of G over m-tiles; the last chunk carries the z payload.
CHUNK_MS = [[0, 1, 2, 3]]
NCHUNK = len(CHUNK_MS)
MC = len(CHUNK_MS[0])     # m-tiles per chunk (2)
NPAIR_C = 4 * MC          # DoubleRow k-tile pairs per chunk (8)
ZCOLS = (BT * C) // MC    # fp32 z cols per row in the last chunk (10)

F32 = mybir.dt.float32
BF16 = mybir.dt.bfloat16
FP8 = mybir.dt.float8e4

LAST_RESULTS = None  # BassKernelResults of the most recent HW run


def ts(i, n):
    return slice(i * n, (i + 1) * n)


def _pair_ks(c, t):
    """Global k-tile indices (k = 4*rank + m) for DoubleRow pair t of chunk c.
    Gathered chunk rows are rank-major, m-minor; pair t covers row-tiles
    (2t, 2t+1) of the gathered output."""
    ms = CHUNK_MS[c]
    ka = (2 * t) // MC * 4 + ms[(2 * t) % MC]
    kb = (2 * t + 1) // MC * 4 + ms[(2 * t + 1) % MC]
    return ka, kb


def build_kernel():
    nc = bacc.Bacc(
        "TRN2",
        target_bir_lowering=False,
        debug=False,
        enable_asserts=False,
        num_devices=NCORES,
    )

    # ---- I/O (per-core shards prepared on host) -------------------------
    xtg = nc.dram_tensor("xtg", [NG, P, KG * B], BF16, kind="ExternalInput")
    # first k-half: k-group-major (DMA-arrival order); second: m-major
    w1a = nc.dram_tensor("w1a", [NG // 2, P, KG * HC], BF16,
                         kind="ExternalInput")
    w1b = nc.dram_tensor("w1b", [MT, P, 16 * P], BF16, kind="ExternalInput")
    wrg = nc.dram_tensor("wrg", [NCHUNK, NPAIR_C, P, 2, DC], FP8,
                         kind="ExternalInput")
    w2sb = nc.dram_tensor("w2sb", [MT, P, B], BF16, kind="ExternalInput")
    w2c = nc.dram_tensor("w2c", [MT, P, C], F32, kind="ExternalInput")
    b1c = nc.dram_tensor("b1c", [MT, P], F32, kind="ExternalInput")
    yoh = nc.dram_tensor("yoh", [P, BT * C], F32, kind="ExternalInput")
    yohi = nc.dram_tensor("yohi", [P, BT * C], F32, kind="ExternalInput")
    b2bc = nc.dram_tensor("b2bc", [P, BT * C], F32, kind="ExternalInput")
    out = nc.dram_tensor("out", [1, 1], F32, kind="ExternalOutput")

    rg = [list(range(NCORES))]

    with tile.TileContext(nc) as tc:
        with (
            tc.tile_pool(name="dram", bufs=1, space="DRAM") as dpool,
            tc.tile_pool(name="xtp", bufs=NG) as xtp,
            tc.tile_pool(name="w1p", bufs=3) as w1p,
            tc.tile_pool(name="wrp", bufs=NCHUNK * NPAIR_C) as wrp,
            tc.tile_pool(name="gfp", bufs=8) as gfp,
            tc.tile_pool(name="w2sp", bufs=MT) as w2sp,
            tc.tile_pool(name="gmp", bufs=MT) as gmp,
            tc.tile_pool(name="resident", bufs=1) as res,
            tc.tile_pool(name="psA", bufs=4, space="PSUM") as psA,
            tc.tile_pool(name="psZ", bufs=2, space="PSUM") as psZ,
        ):
            # ---- collective bounce buffers in DRAM -----------------------
            g_in, g_out = [], []
            for c in range(NCHUNK):
                g_in.append(dpool.tile([MC * P, B], FP8, name=f"g_in{c}"))
                g_out.append(dpool.tile([NCORES * MC * P, B], FP8,
                                        name=f"g_out{c}", addr_space="Shared"))
            # z partials travel in a bf16 container (fp32 bitcast): fp8
            # containers corrupt arbitrary byte payloads (denormal flush)
            z_in = dpool.tile([P, 2 * BT * C], BF16, name="z_in")
            z_out = dpool.tile([NCORES * P, 2 * BT * C], BF16,
                               name="z_out", addr_space="Shared")
            w_in = dpool.tile([B], F32, name="w_in")
            w_out = dpool.tile([B], F32, name="w_out", addr_space="Shared")
            warm_in = dpool.tile([1, 8], F32, name="warm_in")
            warm_out = dpool.tile([NCORES, 8], F32, name="warm_out",
                                  addr_space="Shared")

            # warmup collective: starts the CC engine's ~50us init early so
            # it overlaps mm1 (bypass AllGather = cheapest op type on CC)
            warm_sb = res.tile([1, 8], F32, name="warm_sb")
            nc.vector.memset(warm_sb[:], 0.0)
            nc.gpsimd.dma_start(warm_in[:], warm_sb[:])
            nc.gpsimd.collective_compute(
                "AllGather", mybir.AluOpType.bypass,
                replica_groups=rg, ins=[warm_in.opt()], outs=[warm_out.opt()],
            )

            # ---- constants (gpsimd SW queue, off the critical streams) --
            ones_col = res.tile([P, 1], F32, name="ones_col")
            nc.vector.memset(ones_col[:], 1.0)
            eps8 = res.tile([P, 1], F32, name="eps8")
            nc.vector.memset(eps8[:], 1e-8)
            b2bt = res.tile([P, BT * C], F32, name="b2bt")
            nc.gpsimd.dma_start(b2bt[:], b2bc[:])
            b1t = res.tile([P, MT], F32, name="b1t")
            for m in range(MT):
                nc.gpsimd.dma_start(b1t[:, m : m + 1], b1c[m])
            yoht = res.tile([P, BT * C], F32, name="yoht")
            yohit = res.tile([P, BT * C], F32, name="yohit")
            nc.gpsimd.dma_start(yoht[:], yoh[:])
            nc.gpsimd.dma_start(yohit[:], yohi[:])
            w2t = res.tile([P, MT * C], F32, name="w2t")  # [128, (m,c)]
            for m in range(MT):
                nc.gpsimd.dma_start(w2t[:, ts(m, C)], w2c[m])
            w2s_t = []
            for m in range(MT):
                w2s_m = w2sp.tile([P, B], BF16, name="w2s_m", tag="w2s")
                nc.gpsimd.dma_start(w2s_m[:], w2sb[m])
                w2s_t.append(w2s_m)

            # ---- weight / activation streams ---------------------------
            xt_t = []
            for g in range(NG):
                xt_g = xtp.tile([P, KG * B], BF16, name="xt_g", tag="xt")
                nc.sync.dma_start(xt_g[:], xtg[g])
                xt_t.append(xt_g)
            # mm3 W1T pair tiles: chunk A's on sync (behind xt), chunk B's
            # on scalar (queued behind the mm1 w1 stream below)
            wr_t = [[None] * NPAIR_C for _ in range(NCHUNK)]
            for t in range(NPAIR_C):
                wr = wrp.tile([P, 2, DC], FP8, name=f"wrA_{t}", tag="wr")
                nc.sync.dma_start(wr[:], wrg[0, t])
                wr_t[0][t] = wr

            # ---- mm1: aT[m] = (x @ W1_c).T ------------------------------
            ps_a = [psA.tile([P, B], F32, name=f"ps_a{m}", tag="psA")
                    for m in range(MT)]
            h_t = [None] * MT
            for g in range(NG // 2):
                w1_t = w1p.tile([P, KG * HC], BF16, name="w1a_t", tag="w1")
                nc.scalar.dma_start(w1_t[:], w1a[g])
                for i in range(KG):
                    k = g * KG + i
                    for m in range(MT):
                        nc.tensor.matmul(
                            ps_a[m][:],
                            w1_t[:, i * HC + m * P : i * HC + (m + 1) * P],
                            xt_t[g][:, ts(i, B)],
                            start=(k == 0),
                            stop=False,
                        )
            for m in range(MT):
                w1_t = w1p.tile([P, 16 * P], BF16, name="w1b_t", tag="w1")
                nc.scalar.dma_start(w1_t[:], w1b[m])
                for i in range(16):
                    k = 16 + i
                    g, ii = divmod(k, KG)
                    nc.tensor.matmul(
                        ps_a[m][:],
                        w1_t[:, ts(i, P)],
                        xt_t[g][:, ts(ii, B)],
                        start=False,
                        stop=(i == 15),
                    )
                # h = relu(a + b1); G = (h > 0) * w2sel -> fp8
                h_m = res.tile([P, B], F32, name=f"h_{m}")
                nc.vector.tensor_scalar(
                    h_m[:], ps_a[m][:], b1t[:, m : m + 1], 0.0,
                    op0=mybir.AluOpType.add, op1=mybir.AluOpType.max,
                )
                h_t[m] = h_m
                g_m = gmp.tile([P, B], FP8, name="g_m", tag="g")
                nc.vector.scalar_tensor_tensor(
                    g_m[:], h_m[:], 0.0, w2s_t[m][:],
                    op0=mybir.AluOpType.is_gt,
                    op1=mybir.AluOpType.mult,
                )
                c = m // MC
                nc.gpsimd.dma_start(g_in[c][ts(m % MC, P), 0:B], g_m[:])
                # kick each chunk as soon as its G tiles are written
                if m == CHUNK_MS[c][-1]:
                    nc.gpsimd.collective_compute(
                        "AllGather", mybir.AluOpType.bypass,
                        replica_groups=rg,
                        ins=[g_in[c].opt()], outs=[g_out[c].opt()],
                    )

            # mm3 W1T pair tiles for later chunks (scalar, after w1 stream)
            for c in range(1, NCHUNK):
                for t in range(NPAIR_C):
                    wr = wrp.tile([P, 2, DC], FP8, name=f"wrB_{c}_{t}",
                                  tag="wr")
                    nc.scalar.dma_start(wr[:], wrg[c, t])
                    wr_t[c][t] = wr

            # ---- mm2: z.T partial [b, c] (PE, right after mm1) ----------
            zt_sb = res.tile([P, BT * C], F32, name="zt_sb")
            for t in range(BT):
                ps_z = psZ.tile([P, C], F32, name="ps_z", tag="psZ")
                for m in range(MT):
                    nc.tensor.matmul(
                        ps_z[:], h_t[m][:, ts(t, P)], w2t[:, ts(m, C)],
                        start=(m == 0), stop=(m == MT - 1),
                    )
                nc.vector.tensor_copy(zt_sb[:, ts(t, C)], ps_z[:])

            # pack fp32 z-partials into the bf16 container and AllGather
            nc.gpsimd.dma_start(z_in.bitcast(F32)[:, 0 : BT * C], zt_sb[:])
            nc.gpsimd.collective_compute(
                "AllGather", mybir.AluOpType.bypass,
                replica_groups=rg,
                ins=[z_in.opt()], outs=[z_out.opt()],
            )

            # ---- unpack + sum z partials (overlaps mm3) -----------------
            zacc8 = res.tile([P, BT * C * NCORES], F32, name="zacc8")
            z_out_f = z_out.bitcast(F32)
            for r in range(NCORES):
                nc.scalar.dma_start(
                    zacc8[:, ts(r, BT * C)],
                    z_out_f[ts(r, P), 0 : BT * C],
                )
            zp4 = res.tile([P, BT * C * 4], F32, name="zp4")
            nc.vector.tensor_add(
                zp4[:], zacc8[:, : BT * C * 4], zacc8[:, BT * C * 4 :]
            )
            zp2 = res.tile([P, BT * C * 2], F32, name="zp2")
            nc.vector.tensor_add(
                zp2[:], zp4[:, : BT * C * 2], zp4[:, BT * C * 2 :]
            )
            zf0 = res.tile([P, BT * C], F32, name="zf0")
            nc.vector.tensor_add(zf0[:], zp2[:, : BT * C], zp2[:, BT * C :])
            zf = res.tile([P, BT * C], F32, name="zf")
            nc.vector.tensor_add(zf[:], zf0[:], b2bt[:])

            # ---- z-derived loss stats (overlap with mm3) ----------------
            S = res.tile([P, 16], F32, name="S")
            zf3 = zf[:].rearrange("p (t c) -> p t c", c=C)

            dz = res.tile([P, BT * C], F32, name="dz")
            nc.vector.tensor_sub(dz[:], zf[:], yoht[:])
            dz2 = res.tile([P, BT * C], F32, name="dz2")
            nc.vector.tensor_mul(dz2[:], dz[:], dz[:])
            nc.vector.reduce_sum(
                S[:, 0:4], dz2[:].rearrange("p (t c) -> p t c", c=C),
                axis=mybir.AxisListType.X,
            )
            zyh = res.tile([P, BT * C], F32, name="zyh")
            nc.vector.tensor_mul(zyh[:], zf[:], yoht[:])
            zy = res.tile([P, BT], F32, name="zy")
            nc.vector.reduce_sum(
                zy[:], zyh[:].rearrange("p (t c) -> p t c", c=C),
                axis=mybir.AxisListType.X,
            )
            zmax = res.tile([P, BT], F32, name="zmax")
            nc.vector.reduce_max(zmax[:], zf3, axis=mybir.AxisListType.X)
            nc.vector.tensor_tensor(
                S[:, 8:12], zy[:], zmax[:], op=mybir.AluOpType.is_ge
            )
            omz = res.tile([P, BT], F32, name="omz")
            nc.vector.tensor_scalar(
                omz[:], zy[:], -1.0, 1.0,
                op0=mybir.AluOpType.mult, op1=mybir.AluOpType.add,
            )
            mg = res.tile([P, BT * C], F32, name="mg")
            for t in range(BT):
                nc.scalar.activation(
                    mg[:, ts(t, C)], zf[:, ts(t, C)],
                    mybir.ActivationFunctionType.Relu,
                    bias=omz[:, t : t + 1],
                )
            mgm = res.tile([P, BT * C], F32, name="mgm")
            nc.vector.tensor_mul(mgm[:], mg[:], yohit[:])
            nc.vector.reduce_sum(
                S[:, 4:8], mgm[:].rearrange("p (t c) -> p t c", c=C),
                axis=mybir.AxisListType.X,
            )
            den = res.tile([P, BT], F32, name="den")
            nc.scalar.activation(den[:], zy[:],
                                 mybir.ActivationFunctionType.Abs,
                                 bias=eps8[:, 0:1])
            rec = res.tile([P, BT], F32, name="rec")
            nc.vector.reciprocal(rec[:], den[:])

            # ---- mm3: grads[b, d_local] = G @ W1_c.T, fp8 DoubleRow -----
            ps_g = [psA.tile([P, DC], F32, name=f"ps_g{m}", tag="psA")
                    for m in range(BT)]
            npair = 0
            for c in range(NCHUNK):
                for t in range(NPAIR_C):
                    gf2 = gfp.tile([P, 2, B], FP8, name="gf2", tag="gf")
                    nc.sync.dma_start(
                        gf2[:],
                        g_out[c][ts(t, 2 * P), 0:B].rearrange(
                            "(two p) w -> p two w", p=P),
                    )
                    first = npair == 0
                    last = npair == NCHUNK * NPAIR_C - 1
                    for bt in range(BT):
                        nc.tensor.matmul(
                            ps_g[bt][:],
                            gf2[:, :, ts(bt, P)],
                            wr_t[c][t][:],
                            start=first,
                            stop=last,
                            perf_mode=mybir.MatmulPerfMode.DoubleRow,
                        )
                    npair += 1

            # ---- w_l1 partial: sum_d |grads|, AllReduce over cores ------
            # per-bt reduce + DMA so the AllReduce doorbell fires as soon as
            # the last batch tile's reduce lands
            wl1_p = res.tile([P, BT], F32, name="wl1_p")
            for t in range(BT):
                nc.vector.reduce_sum(
                    wl1_p[:, t : t + 1], ps_g[t][:],
                    axis=mybir.AxisListType.X, apply_absolute_value=True,
                )
                nc.sync.dma_start(w_in[ts(t, P)], wl1_p[:, t : t + 1])
            nc.gpsimd.collective_compute(
                "AllReduce", mybir.AluOpType.add,
                replica_groups=rg, ins=[w_in.opt()], outs=[w_out.opt()],
            )
            wl1 = res.tile([P, BT], F32, name="wl1")
            nc.scalar.dma_start(
                wl1[:], w_out.rearrange("(t p) -> p t", p=P)
            )

            # ---- nsr partials -> S[:, 12:16] ---------------------------
            rt2 = res.tile([P, BT], F32, name="rt2")
            nc.vector.scalar_tensor_tensor(
                rt2[:], wl1[:], EPS_EFF, rec[:],
                op0=mybir.AluOpType.mult, op1=mybir.AluOpType.mult,
            )
            nc.scalar.activation(
                S[:, 12:16], rt2[:], mybir.ActivationFunctionType.Ln, bias=1.0
            )

            # ---- final scalar ------------------------------------------
            ps_fin = psZ.tile([1, 16], F32, name="ps_fin", tag="psF")
            nc.tensor.matmul(ps_fin[:], ones_col[:], S[:], start=True,
                             stop=True)
            tots = res.tile([1, 4], F32, name="tots")
            nc.vector.reduce_sum(
                tots[:], ps_fin[:].rearrange("p (s t) -> p s t", t=BT),
                axis=mybir.AxisListType.X,
            )
            # loss = mse/2560 + (margin + BETA*nsr) * corr / (B*B)
            t_a = res.tile([1, 1], F32, name="t_a")
            nc.vector.scalar_tensor_tensor(
                t_a[:], tots[:, 3:4], BETA, tots[:, 1:2],
                op0=mybir.AluOpType.mult, op1=mybir.AluOpType.add,
            )
            t_b = res.tile([1, 1], F32, name="t_b")
            nc.vector.scalar_tensor_tensor(
                t_b[:], t_a[:], 1.0 / (B * B), tots[:, 2:3],
                op0=mybir.AluOpType.mult, op1=mybir.AluOpType.mult,
            )
            t_g = res.tile([1, 1], F32, name="t_g")
            nc.vector.scalar_tensor_tensor(
                t_g[:], tots[:, 0:1], 1.0 / (B * C), t_b[:],
                op0=mybir.AluOpType.mult, op1=mybir.AluOpType.add,
            )
            nc.sync.dma_start(out[:], t_g[:])

    nc.compile()
    return nc


def _pack_ktiles(arr, group=KG):
    """[K*128, N] row-major -> [K/group, 128, group*N] so each grouped DMA
    reads group*N contiguous bytes per partition row."""
    K = arr.shape[0] // P
    N = arr.shape[1]
    return np.ascontiguousarray(
        arr.reshape(K // group, group, P, N).transpose(0, 2, 1, 3).reshape(
            K // group, P, group * N
        )
    )


def prep_inputs(x, y, W1, b1, W2, b2):
    """Host-side shard + layout prep.  Returns in_maps for the 8 cores."""
    x = np.asarray(x, dtype=np.float32)
    y = np.asarray(y).astype(np.int64)
    W1 = np.asarray(W1, dtype=np.float32)
    b1 = np.asarray(b1, dtype=np.float32)
    W2 = np.asarray(W2, dtype=np.float32)
    b2 = np.asarray(b2, dtype=np.float32)
    bf = ml_dtypes.bfloat16
    f8 = ml_dtypes.float8_e4m3fn

    xtg = _pack_ktiles(np.ascontiguousarray(x.T).astype(bf))     # [NG,P,KG*B]
    w2sel_full = W2[:, y]                                        # [H, B]
    yoh = np.zeros((P, BT * C), np.float32)
    for t in range(BT):
        for p in range(P):
            yoh[p, t * C + int(y[t * P + p])] = 1.0
    yohi = (1.0 - yoh).astype(np.float32)
    b2bc = np.ascontiguousarray(
        np.tile(b2.reshape(1, C), (P, BT)).astype(np.float32))

    in_maps = []
    for cid in range(NCORES):
        hs = slice(cid * HC, (cid + 1) * HC)
        ds = slice(cid * DC, (cid + 1) * DC)
        W1c = W1[:, hs]                                          # [D, HC]
        # first k-half: grouped k-major [NG/2, P, KG*HC]
        w1ag = _pack_ktiles(W1c[: D // 2].astype(bf))
        # second k-half: m-major [MT, P, 16*P]
        w1bg = np.ascontiguousarray(
            W1c[D // 2 :].reshape(16, P, MT, P).transpose(2, 1, 0, 3).reshape(
                MT, P, 16 * P)).astype(bf)
        # W1T k-tiles (over H) paired for DoubleRow per AG chunk
        W1T = np.ascontiguousarray(W1[ds, :].T * GSCALE)         # [H, DC]
        W1Tk = W1T.reshape(KT, P, DC)
        wrg = np.zeros((NCHUNK, NPAIR_C, P, 2, DC), np.float32)
        for c in range(NCHUNK):
            for t in range(NPAIR_C):
                ka, kb = _pair_ks(c, t)
                wrg[c, t, :, 0, :] = W1Tk[ka]
                wrg[c, t, :, 1, :] = W1Tk[kb]
        wrg = wrg.astype(f8)
        w2sb = np.ascontiguousarray(
            (w2sel_full[hs, :] * GSCALE).reshape(MT, P, B)).astype(bf)
        in_maps.append({
            "xtg": xtg,
            "w1a": w1ag,
            "w1b": w1bg,
            "wrg": wrg,
            "w2sb": w2sb,
            "w2c": np.ascontiguousarray(W2[hs, :].reshape(MT, P, C)),
            "b1c": np.ascontiguousarray(b1[hs].reshape(MT, P)),
            "yoh": yoh,
            "yohi": yohi,
            "b2bc": b2bc,
        })
    return in_maps


@functools.lru_cache(maxsize=1)
def get_nc():
    return build_kernel()


def kernel(x, y, W1, b1, W2, b2):
    global LAST_RESULTS
    nc = get_nc()
    in_maps = prep_inputs(x, y, W1, b1, W2, b2)

    if os.environ.get("BASSK_SIM"):
        from concourse.bass_interp import MultiCoreSim
        sim = MultiCoreSim(
            nc, num_cores=NCORES, require_finite=False, require_nnan=False
        )
        for c in range(NCORES):
            for k, v in in_maps[c].items():
                sim.cores[c].tensor(k)[:] = v
        sim.simulate(check_with_hw=False)
        res = np.array(sim.cores[0].tensor("out"))
    else:
        r = run_bass_kernel_spmd(
            nc,
            in_maps,
            core_ids=list(range(NCORES)),
            trace=bool(os.environ.get("BASSK_TRACE")),
        )
        LAST_RESULTS = r
        res = r.results[0]["out"]

    return np.float32(res.reshape(())).reshape(())


# revision 22
# speedup vs baseline: 1.0469x; 1.0469x over previous
"""Trainium2 Bass kernel for nn_NSRLossV2 (8-core SPMD).

Math (reference, fp32):
    a  = x @ W1 + b1            [B, H]
    h  = relu(a)
    z  = h @ W2 + b2            [B, C]
    mse    = mean((z - onehot(y))^2)
    margin = sum(relu(1 - z_y + z) * (1 - onehot)) / B
    grads  = ((a > 0) * W2[:, y].T) @ W1.T        [B, D]
    w_l1   = sum_d |grads|
    R      = w_l1 * EPS / (|z_y| + 1e-8)
    nsr    = BETA * mean(log(1 + R))
    acc    = mean(argmax(z) == y)
    loss   = mse + (margin + nsr) * acc

Sharding: model-parallel over H (each core owns a 512-wide slice of W1's
columns).  mm1 in bf16: the first k-half runs k-group-major so the PE
tracks the xt DMA stream, the second half m-major so each 128-row tile
of G = (a>0)*W2[:,y] completes early.  G is cast to fp8 e4m3 (x1024) and
AllGathered in two chunks overlapped with mm1's tail and mm3; fp32 z
partials ride chunk B bitcast into extra byte columns.  mm3 (grads) runs
in fp8 DoubleRow perf mode (2 k-tiles per matmul).  Per-sample |grads|
L1 partials are AllReduced at the end.
"""

import os
import functools

import numpy as np
import ml_dtypes

import concourse.bass as bass
import concourse.bacc as bacc
import concourse.mybir as mybir
import concourse.tile as tile
from concourse.bass_utils import run_bass_kernel_spmd

NCORES = 8
B, D, H, C = 512, 4096, 4096, 5
HC = H // NCORES          # per-core H slice (512)
DC = D // NCORES          # per-core D slice (512)
P = 128
KT = D // P               # 32 k-tiles over D (mm1)
KG = 4                    # k-tiles per grouped xt DMA
NG = KT // KG             # 8 grouped xt loads
MT = HC // P              # 4 m-tiles over the local H slice
BT = B // P               # 4 b-tiles over the batch
BETA, EPS = 0.4, 0.05
GSCALE = 1024.0           # fp8 scale for G and W1T (grads scaled by 2^20)
EPS_EFF = EPS / (GSCALE * GSCALE)

# AllGather chunking of G over m-tiles; the last chunk carries the z payload.
CHUNK_MS = [[0, 1, 2, 3]]
NCHUNK = len(CHUNK_MS)
MC = len(CHUNK_MS[0])     # m-tiles per chunk (2)
NPAIR_C = 4 * MC          # DoubleRow k-tile pairs per chunk (8)
ZCOLS = (BT * C) // MC    # fp32 z cols per row in the last chunk (10)

F32 = mybir.dt.float32
BF16 = mybir.dt.bfloat16
FP8 = mybir.dt.float8e4

LAST_RESULTS = None  # BassKernelResults of the most recent HW run


def ts(i, n):
    return slice(i * n, (i + 1) * n)


def _pair_ks(c, t):
    """Global k-tile indices (k = 4*rank + m) for DoubleRow pair t of chunk c.
    Gathered chunk rows are rank-major, m-minor; pair t covers row-tiles
    (2t, 2t+1) of the gathered output."""
    ms = CHUNK_MS[c]
    ka = (2 * t) // MC * 4 + ms[(2 * t) % MC]
    kb = (2 * t + 1) // MC * 4 + ms[(2 * t + 1) % MC]
    return ka, kb


def build_kernel():
    nc = bacc.Bacc(
        "TRN2",
        target_bir_lowering=False,
        debug=False,
        enable_asserts=False,
        num_devices=NCORES,
    )

    # ---- I/O (per-core shards prepared on host) -------------------------
    xtg = nc.dram_tensor("xtg", [NG, P, KG * B], BF16, kind="ExternalInput")
    # first k-half: k-group-major (DMA-arrival order); second: m-major
    w1a = nc.dram_tensor("w1a", [NG // 2, P, KG * HC], BF16,
                         kind="ExternalInput")
    w1b = nc.dram_tensor("w1b", [MT, P, 16 * P], BF16, kind="ExternalInput")
    wrg = nc.dram_tensor("wrg", [NCHUNK, NPAIR_C, P, 2, DC], FP8,
                         kind="ExternalInput")
    w2sb = nc.dram_tensor("w2sb", [MT, P, B], BF16, kind="ExternalInput")
    w2c = nc.dram_tensor("w2c", [MT, P, C], F32, kind="ExternalInput")
    b1c = nc.dram_tensor("b1c", [MT, P], F32, kind="ExternalInput")
    yoh = nc.dram_tensor("yoh", [P, BT * C], F32, kind="ExternalInput")
    yohi = nc.dram_tensor("yohi", [P, BT * C], F32, kind="ExternalInput")
    b2bc = nc.dram_tensor("b2bc", [P, BT * C], F32, kind="ExternalInput")
    out = nc.dram_tensor("out", [1, 1], F32, kind="ExternalOutput")

    rg = [list(range(NCORES))]

    with tile.TileContext(nc) as tc:
        with (
            tc.tile_pool(name="dram", bufs=1, space="DRAM") as dpool,
            tc.tile_pool(name="xtp", bufs=NG) as xtp,
            tc.tile_pool(name="w1p", bufs=3) as w1p,
            tc.tile_pool(name="wrp", bufs=NCHUNK * NPAIR_C) as wrp,
            tc.tile_pool(name="gfp", bufs=8) as gfp,
            tc.tile_pool(name="w2sp", bufs=MT) as w2sp,
            tc.tile_pool(name="gmp", bufs=MT) as gmp,
            tc.tile_pool(name="resident", bufs=1) as res,
            tc.tile_pool(name="psA", bufs=4, space="PSUM") as psA,
            tc.tile_pool(name="psZ", bufs=2, space="PSUM") as psZ,
        ):
            # ---- collective bounce buffers in DRAM -----------------------
            g_in, g_out = [], []
            for c in range(NCHUNK):
                g_in.append(dpool.tile([MC * P, B], FP8, name=f"g_in{c}"))
                g_out.append(dpool.tile([NCORES * MC * P, B], FP8,
                                        name=f"g_out{c}", addr_space="Shared"))
            # z partials travel in a bf16 container (fp32 bitcast): fp8
            # containers corrupt arbitrary byte payloads (denormal flush)
            z_in = dpool.tile([P, 2 * BT * C], BF16, name="z_in")
            z_out = dpool.tile([NCORES * P, 2 * BT * C], BF16,
                               name="z_out", addr_space="Shared")
            w_in = dpool.tile([B], F32, name="w_in")
            w_out = dpool.tile([B], F32, name="w_out", addr_space="Shared")
            warm_in = dpool.tile([1, 8], F32, name="warm_in")
            warm_out = dpool.tile([1, 8], F32, name="warm_out",
                                  addr_space="Shared")

            # warmup collective: starts the CC engine's ~50us init early so
            # it overlaps mm1
            warm_sb = res.tile([1, 8], F32, name="warm_sb")
            nc.vector.memset(warm_sb[:], 0.0)
            nc.sync.dma_start(warm_in[:], warm_sb[:])
            nc.gpsimd.collective_compute(
                "AllReduce", mybir.AluOpType.add,
                replica_groups=rg, ins=[warm_in.opt()], outs=[warm_out.opt()],
            )

            # ---- constants (gpsimd SW queue, off the critical streams) --
            ones_col = res.tile([P, 1], F32, name="ones_col")
            nc.vector.memset(ones_col[:], 1.0)
            eps8 = res.tile([P, 1], F32, name="eps8")
            nc.vector.memset(eps8[:], 1e-8)
            b2bt = res.tile([P, BT * C], F32, name="b2bt")
            nc.gpsimd.dma_start(b2bt[:], b2bc[:])
            b1t = res.tile([P, MT], F32, name="b1t")
            for m in range(MT):
                nc.gpsimd.dma_start(b1t[:, m : m + 1], b1c[m])
            yoht = res.tile([P, BT * C], F32, name="yoht")
            yohit = res.tile([P, BT * C], F32, name="yohit")
            nc.gpsimd.dma_start(yoht[:], yoh[:])
            nc.gpsimd.dma_start(yohit[:], yohi[:])
            w2t = res.tile([P, MT * C], F32, name="w2t")  # [128, (m,c)]
            for m in range(MT):
                nc.gpsimd.dma_start(w2t[:, ts(m, C)], w2c[m])
            w2s_t = []
            for m in range(MT):
                w2s_m = w2sp.tile([P, B], BF16, name="w2s_m", tag="w2s")
                nc.gpsimd.dma_start(w2s_m[:], w2sb[m])
                w2s_t.append(w2s_m)

            # ---- weight / activation streams ---------------------------
            xt_t = []
            for g in range(NG):
                xt_g = xtp.tile([P, KG * B], BF16, name="xt_g", tag="xt")
                nc.sync.dma_start(xt_g[:], xtg[g])
                xt_t.append(xt_g)
            # mm3 W1T pair tiles: chunk A's on sync (behind xt), chunk B's
            # on scalar (queued behind the mm1 w1 stream below)
            wr_t = [[None] * NPAIR_C for _ in range(NCHUNK)]
            for t in range(NPAIR_C):
                wr = wrp.tile([P, 2, DC], FP8, name=f"wrA_{t}", tag="wr")
                nc.sync.dma_start(wr[:], wrg[0, t])
                wr_t[0][t] = wr

            # ---- mm1: aT[m] = (x @ W1_c).T ------------------------------
            ps_a = [psA.tile([P, B], F32, name=f"ps_a{m}", tag="psA")
                    for m in range(MT)]
            h_t = [None] * MT
            for g in range(NG // 2):
                w1_t = w1p.tile([P, KG * HC], BF16, name="w1a_t", tag="w1")
                nc.scalar.dma_start(w1_t[:], w1a[g])
                for i in range(KG):
                    k = g * KG + i
                    for m in range(MT):
                        nc.tensor.matmul(
                            ps_a[m][:],
                            w1_t[:, i * HC + m * P : i * HC + (m + 1) * P],
                            xt_t[g][:, ts(i, B)],
                            start=(k == 0),
                            stop=False,
                        )
            for m in range(MT):
                w1_t = w1p.tile([P, 16 * P], BF16, name="w1b_t", tag="w1")
                nc.scalar.dma_start(w1_t[:], w1b[m])
                for i in range(16):
                    k = 16 + i
                    g, ii = divmod(k, KG)
                    nc.tensor.matmul(
                        ps_a[m][:],
                        w1_t[:, ts(i, P)],
                        xt_t[g][:, ts(ii, B)],
                        start=False,
                        stop=(i == 15),
                    )
                # h = relu(a + b1); G = (h > 0) * w2sel -> fp8
                h_m = res.tile([P, B], F32, name=f"h_{m}")
                nc.vector.tensor_scalar(
                    h_m[:], ps_a[m][:], b1t[:, m : m + 1], 0.0,
                    op0=mybir.AluOpType.add, op1=mybir.AluOpType.max,
                )
                h_t[m] = h_m
                g_m = gmp.tile([P, B], FP8, name="g_m", tag="g")
                nc.vector.scalar_tensor_tensor(
                    g_m[:], h_m[:], 0.0, w2s_t[m][:],
                    op0=mybir.AluOpType.is_gt,
                    op1=mybir.AluOpType.mult,
                )
                c = m // MC
                nc.gpsimd.dma_start(g_in[c][ts(m % MC, P), 0:B], g_m[:])
                # kick each chunk as soon as its G tiles are written
                if m == CHUNK_MS[c][-1]:
                    nc.gpsimd.collective_compute(
                        "AllGather", mybir.AluOpType.bypass,
                        replica_groups=rg,
                        ins=[g_in[c].opt()], outs=[g_out[c].opt()],
                    )

            # mm3 W1T pair tiles for later chunks (scalar, after w1 stream)
            for c in range(1, NCHUNK):
                for t in range(NPAIR_C):
                    wr = wrp.tile([P, 2, DC], FP8, name=f"wrB_{c}_{t}",
                                  tag="wr")
                    nc.scalar.dma_start(wr[:], wrg[c, t])
                    wr_t[c][t] = wr

            # ---- mm2: z.T partial [b, c] (PE, right after mm1) ----------
            zt_sb = res.tile([P, BT * C], F32, name="zt_sb")
            for t in range(BT):
                ps_z = psZ.tile([P, C], F32, name="ps_z", tag="psZ")
                for m in range(MT):
                    nc.tensor.matmul(
                        ps_z[:], h_t[m][:, ts(t, P)], w2t[:, ts(m, C)],
                        start=(m == 0), stop=(m == MT - 1),
                    )
                nc.vector.tensor_copy(zt_sb[:, ts(t, C)], ps_z[:])

            # pack fp32 z-partials into the bf16 container and AllGather
            nc.gpsimd.dma_start(z_in.bitcast(F32)[:, 0 : BT * C], zt_sb[:])
            nc.gpsimd.collective_compute(
                "AllGather", mybir.AluOpType.bypass,
                replica_groups=rg,
                ins=[z_in.opt()], outs=[z_out.opt()],
            )

            # ---- unpack + sum z partials (overlaps mm3) -----------------
            zacc8 = res.tile([P, BT * C * NCORES], F32, name="zacc8")
            z_out_f = z_out.bitcast(F32)
            for r in range(NCORES):
                nc.scalar.dma_start(
                    zacc8[:, ts(r, BT * C)],
                    z_out_f[ts(r, P), 0 : BT * C],
                )
            zp4 = res.tile([P, BT * C * 4], F32, name="zp4")
            nc.vector.tensor_add(
                zp4[:], zacc8[:, : BT * C * 4], zacc8[:, BT * C * 4 :]
            )
            zp2 = res.tile([P, BT * C * 2], F32, name="zp2")
            nc.vector.tensor_add(
                zp2[:], zp4[:, : BT * C * 2], zp4[:, BT * C * 2 :]
            )
            zf0 = res.tile([P, BT * C], F32, name="zf0")
            nc.vector.tensor_add(zf0[:], zp2[:, : BT * C], zp2[:, BT * C :])
            zf = res.tile([P, BT * C], F32, name="zf")
            nc.vector.tensor_add(zf[:], zf0[:], b2bt[:])

            # ---- z-derived loss stats (overlap with mm3) ----------------
            S = res.tile([P, 16], F32, name="S")
            zf3 = zf[:].rearrange("p (t c) -> p t c", c=C)

            dz = res.tile([P, BT * C], F32, name="dz")
            nc.vector.tensor_sub(dz[:], zf[:], yoht[:])
            dz2 = res.tile([P, BT * C], F32, name="dz2")
            nc.vector.tensor_mul(dz2[:], dz[:], dz[:])
            nc.vector.reduce_sum(
                S[:, 0:4], dz2[:].rearrange("p (t c) -> p t c", c=C),
                axis=mybir.AxisListType.X,
            )
            zyh = res.tile([P, BT * C], F32, name="zyh")
            nc.vector.tensor_mul(zyh[:], zf[:], yoht[:])
            zy = res.tile([P, BT], F32, name="zy")
            nc.vector.reduce_sum(
                zy[:], zyh[:].rearrange("p (t c) -> p t c", c=C),
                axis=mybir.AxisListType.X,
            )
            zmax = res.tile([P, BT], F32, name="zmax")
            nc.vector.reduce_max(zmax[:], zf3, axis=mybir.AxisListType.X)
            nc.vector.tensor_tensor(
                S[:, 8:12], zy[:], zmax[:], op=mybir.AluOpType.is_ge
            )
            omz = res.tile([P, BT], F32, name="omz")
            nc.vector.tensor_scalar(
                omz[:], zy[:], -1.0, 1.0,
                op0=mybir.AluOpType.mult, op1=mybir.AluOpType.add,
            )
            mg = res.tile([P, BT * C], F32, name="mg")
            for t in range(BT):
                nc.scalar.activation(
                    mg[:, ts(t, C)], zf[:, ts(t, C)],
                    mybir.ActivationFunctionType.Relu,
                    bias=omz[:, t : t + 1],
                )
            mgm = res.tile([P, BT * C], F32, name="mgm")
            nc.vector.tensor_mul(mgm[:], mg[:], yohit[:])
            nc.vector.reduce_sum(
                S[:, 4:8], mgm[:].rearrange("p (t c) -> p t c", c=C),
                axis=mybir.AxisListType.X,
            )
            den = res.tile([P, BT], F32, name="den")
            nc.scalar.activation(den[:], zy[:],
                                 mybir.ActivationFunctionType.Abs,
                                 bias=eps8[:, 0:1])
            rec = res.tile([P, BT], F32, name="rec")
            nc.vector.reciprocal(rec[:], den[:])

            # ---- mm3: grads[b, d_local] = G @ W1_c.T, fp8 DoubleRow -----
            ps_g = [psA.tile([P, DC], F32, name=f"ps_g{m}", tag="psA")
                    for m in range(BT)]
            npair = 0
            for c in range(NCHUNK):
                for t in range(NPAIR_C):
                    gf2 = gfp.tile([P, 2, B], FP8, name="gf2", tag="gf")
                    nc.sync.dma_start(
                        gf2[:],
                        g_out[c][ts(t, 2 * P), 0:B].rearrange(
                            "(two p) w -> p two w", p=P),
                    )
                    first = npair == 0
                    last = npair == NCHUNK * NPAIR_C - 1
                    for bt in range(BT):
                        nc.tensor.matmul(
                            ps_g[bt][:],
                            gf2[:, :, ts(bt, P)],
                            wr_t[c][t][:],
                            start=first,
                            stop=last,
                            perf_mode=mybir.MatmulPerfMode.DoubleRow,
                        )
                    npair += 1

            # ---- w_l1 partial: sum_d |grads|, AllReduce over cores ------
            wl1_p = res.tile([P, BT], F32, name="wl1_p")
            for t in range(BT):
                nc.vector.reduce_sum(
                    wl1_p[:, t : t + 1], ps_g[t][:],
                    axis=mybir.AxisListType.X, apply_absolute_value=True,
                )
            nc.sync.dma_start(
                w_in.rearrange("(t p) -> p t", p=P), wl1_p[:]
            )
            nc.gpsimd.collective_compute(
                "AllReduce", mybir.AluOpType.add,
                replica_groups=rg, ins=[w_in.opt()], outs=[w_out.opt()],
            )
            wl1 = res.tile([P, BT], F32, name="wl1")
            nc.scalar.dma_start(
                wl1[:], w_out.rearrange("(t p) -> p t", p=P)
            )

            # ---- nsr partials -> S[:, 12:16] ---------------------------
            rt2 = res.tile([P, BT], F32, name="rt2")
            nc.vector.scalar_tensor_tensor(
                rt2[:], wl1[:], EPS_EFF, rec[:],
                op0=mybir.AluOpType.mult, op1=mybir.AluOpType.mult,
            )
            nc.scalar.activation(
                S[:, 12:16], rt2[:], mybir.ActivationFunctionType.Ln, bias=1.0
            )

            # ---- final scalar ------------------------------------------
            ps_fin = psZ.tile([1, 16], F32, name="ps_fin", tag="psF")
            nc.tensor.matmul(ps_fin[:], ones_col[:], S[:], start=True,
                             stop=True)
            tots = res.tile([1, 4], F32, name="tots")
            nc.vector.reduce_sum(
                tots[:], ps_fin[:].rearrange("p (s t) -> p s t", t=BT),
                axis=mybir.AxisListType.X,
            )
            # loss = mse/2560 + (margin + BETA*nsr) * corr / (B*B)
            t_a = res.tile([1, 1], F32, name="t_a")
            nc.vector.scalar_tensor_tensor(
                t_a[:], tots[:, 3:4], BETA, tots[:, 1:2],
                op0=mybir.AluOpType.mult, op1=mybir.AluOpType.add,
            )
            t_b = res.tile([1, 1], F32, name="t_b")
            nc.vector.scalar_tensor_tensor(
                t_b[:], t_a[:], 1.0 / (B * B), tots[:, 2:3],
                op0=mybir.AluOpType.mult, op1=mybir.AluOpType.mult,
            )
            t_g = res.tile([1, 1], F32, name="t_g")
            nc.vector.scalar_tensor_tensor(
                t_g[:], tots[:, 0:1], 1.0 / (B * C), t_b[:],
                op0=mybir.AluOpType.mult, op1=mybir.AluOpType.add,
            )
            nc.sync.dma_start(out[:], t_g[:])

    nc.compile()
    return nc


def _pack_ktiles(arr, group=KG):
    """[K*128, N] row-major -> [K/group, 128, group*N] so each grouped DMA
    reads group*N contiguous bytes per partition row."""
    K = arr.shape[0] // P
    N = arr.shape[1]
    return np.ascontiguousarray(
        arr.reshape(K // group, group, P, N).transpose(0, 2, 1, 3).reshape(
            K // group, P, group * N
        )
    )


def prep_inputs(x, y, W1, b1, W2, b2):
    """Host-side shard + layout prep.  Returns in_maps for the 8 cores."""
    x = np.asarray(x, dtype=np.float32)
    y = np.asarray(y).astype(np.int64)
    W1 = np.asarray(W1, dtype=np.float32)
    b1 = np.asarray(b1, dtype=np.float32)
    W2 = np.asarray(W2, dtype=np.float32)
    b2 = np.asarray(b2, dtype=np.float32)
    bf = ml_dtypes.bfloat16
    f8 = ml_dtypes.float8_e4m3fn

    xtg = _pack_ktiles(np.ascontiguousarray(x.T).astype(bf))     # [NG,P,KG*B]
    w2sel_full = W2[:, y]                                        # [H, B]
    yoh = np.zeros((P, BT * C), np.float32)
    for t in range(BT):
        for p in range(P):
            yoh[p, t * C + int(y[t * P + p])] = 1.0
    yohi = (1.0 - yoh).astype(np.float32)
    b2bc = np.ascontiguousarray(
        np.tile(b2.reshape(1, C), (P, BT)).astype(np.float32))

    in_maps = []
    for cid in range(NCORES):
        hs = slice(cid * HC, (cid + 1) * HC)
        ds = slice(cid * DC, (cid + 1) * DC)
        W1c = W1[:, hs]                                          # [D, HC]
        # first k-half: grouped k-major [NG/2, P, KG*HC]
        w1ag = _pack_ktiles(W1c[: D // 2].astype(bf))
        # second k-half: m-major [MT, P, 16*P]
        w1bg = np.ascontiguousarray(
            W1c[D // 2 :].reshape(16, P, MT, P).transpose(2, 1, 0, 3).reshape(
                MT, P, 16 * P)).astype(bf)
        # W1T k-tiles (over H) paired for DoubleRow per AG chunk
        W1T = np.ascontiguousarray(W1[ds, :].T * GSCALE)         # [H, DC]
        W1Tk = W1T.reshape(KT, P, DC)
        wrg = np.zeros((NCHUNK, NPAIR_C, P, 2, DC), np.float32)
        for c in range(NCHUNK):
            for t in range(NPAIR_C):
                ka, kb = _pair_ks(c, t)
                wrg[c, t, :, 0, :] = W1Tk[ka]
                wrg[c, t, :, 1, :] = W1Tk[kb]
        wrg = wrg.astype(f8)
        w2sb = np.ascontiguousarray(
            (w2sel_full[hs, :] * GSCALE).reshape(MT, P, B)).astype(bf)
        in_maps.append({
            "xtg": xtg,
            "w1a": w1ag,
            "w1b": w1bg,
            "wrg": wrg,
            "w2sb": w2sb,
            "w2c": np.ascontiguousarray(W2[hs, :].reshape(MT, P, C)),
            "b1c": np.ascontiguousarray(b1[hs].reshape(MT, P)),
            "yoh": yoh,
            "yohi": yohi,
            "b2bc": b2bc,
        })
    return in_maps


@functools.lru_cache(maxsize=1)
def get_nc():
    return build_kernel()


def kernel(x, y, W1, b1, W2, b2):
    global LAST_RESULTS
    nc = get_nc()
    in_maps = prep_inputs(x, y, W1, b1, W2, b2)

    if os.environ.get("BASSK_SIM"):
        from concourse.bass_interp import MultiCoreSim
        sim = MultiCoreSim(
            nc, num_cores=NCORES, require_finite=False, require_nnan=False
        )
        for c in range(NCORES):
            for k, v in in_maps[c].items():
                sim.cores[c].tensor(k)[:] = v
        sim.simulate(check_with_hw=False)
        res = np.array(sim.cores[0].tensor("out"))
    else:
        r = run_bass_kernel_spmd(
            nc,
            in_maps,
            core_ids=list(range(NCORES)),
            trace=bool(os.environ.get("BASSK_TRACE")),
        )
        LAST_RESULTS = r
        res = r.results[0]["out"]

    return np.float32(res.reshape(())).reshape(())
